# revision 30
# baseline (speedup 1.0000x reference)
"""Trainium2 Bass kernel: gradient of the EnergyAttention scalar energy.

reference:
    q = einsum('bqd,hzd->bqhz', g, wq); k = einsum('bkd,hzd->bkhz', g, wk)
    scores = einsum('bqhz,bkhz->bhqk', q, k)
    E = -(logsumexp(BETA*scores, -1)/BETA).sum() + POS_SCALE*(g*pos).sum()
    out = dE/dg

Math: with P = softmax(BETA*scores) per (b,h,q):
    out[b] = -sum_h [ (P@K) @ wq_h + (P.T@Qn) @ wk_h ] + POS_SCALE*pos
where Qn = diag(1/Z) Q (row-normalized by the softmax partition Z).

Sharding: 8 cores; core c handles batch b=c//4 and heads 4*(c%4)..4*(c%4)+3
(two head-pairs packed into the 128-partition dim).  Each core takes its
inputs in NATURAL layout (x[b], wq[h0:h0+4], wk[h0:h0+4] -- pure views, no
host-side packing) and emits the positive partial sum_h[(dQ)wq + (dK)wk]
in f32; the host negates, sums the four per-core partials of each batch,
and adds the positional term.

Per-core structure (fused pipeline; P tiles are consumed by the transposed
projection matmuls as soon as they are exp'd, so no full [S,S] matrix is
ever materialized):
  prep:   G^T, wq^T, wk^T built on-device via PE transpose-mode
  proj:   QT2/KT2 [z2, s] = (W GT) via f32r matmuls (d contracted in 8 tiles)
  trans:  Qraw/K2n [s, z2] via PE transpose-mode
  loop i: scores/scoresT blocks (row-tiled K=64 pairs, f32r)
          -> exp on ACT (fp16 out, fused row-sum accum for Z)
          -> dK^T += (Q/Z)^T-block @ P-block, dQ^T += K-block @ PT-block
             (col-tiled M=64 pairs, fp16, single has_written group per bank)
  out:    partial = sum_pairs dQT^T wq + dKT^T wk  (K=128-packed f32r)
"""

import threading

import numpy as np

B = 2
S = 1024
D = 1024
NH = 16
Z = 64
BETA = 1.0 / np.sqrt(np.float32(Z))
POS_SCALE = 0.001
N_CORES = 8
HPC = 4           # heads per core
NPAIR = 2         # head pairs per core
ND = D // 128     # 8 d-tiles
NQ = S // 128     # 8 q/k blocks
NCH = S // 512    # 2 moving-dim chunks

_CACHE = {}


def build_nc(reps=1, taps=(), out_mode="f32copy"):
    """Build the (SPMD, identical-per-core) Bass program.

    reps>1 repeats the whole computation (idempotent) inside one NEFF --
    used to measure steady-state per-execution time as a marginal cost.
    taps: debug-only -- names of internal tiles to DMA out as extra outputs.
    out_mode: "f32copy" (default -- positive partial, host negates; the
    baseline-proven evacuation).  fp16 external outputs are corrupted by a
    f32->f16 conversion misapplied to packed f16 pairs somewhere in the
    output plumbing (odd columns bit-exact, even columns zeroed/exponent-
    shifted), so "f16neg"/"f16copy" are debug-only."""
    from contextlib import ExitStack

    import concourse.mybir as mybir
    import concourse.tile as tile
    from concourse import bacc
    from concourse.masks import make_identity

    F32 = mybir.dt.float32
    F32R = mybir.dt.float32r
    F16 = mybir.dt.float16
    MUL = mybir.AluOpType.mult
    EXP = mybir.ActivationFunctionType.Exp
    COPY = mybir.ActivationFunctionType.Copy

    nc = bacc.Bacc(
        "TRN2",
        target_bir_lowering=False,
        debug=False,
        enable_asserts=False,
        num_devices=N_CORES,
    )

    OUT_DT = F32 if out_mode.startswith("f32") else F16
    xb = nc.dram_tensor("xb", [S, D], F32R, kind="ExternalInput").ap()
    w4q = nc.dram_tensor("w4q", [NPAIR * 128, D], F32R, kind="ExternalInput").ap()
    w4k = nc.dram_tensor("w4k", [NPAIR * 128, D], F32R, kind="ExternalInput").ap()
    gout = nc.dram_tensor("gout", [S, D], OUT_DT, kind="ExternalOutput").ap()

    with tile.TileContext(nc) as tc, ExitStack() as ctx:
        sb1 = ctx.enter_context(tc.tile_pool(name="sb1", bufs=1))
        sb2 = ctx.enter_context(tc.tile_pool(name="sb2", bufs=2))
        sb4 = ctx.enter_context(tc.tile_pool(name="sb4", bufs=4))
        pp = ctx.enter_context(tc.tile_pool(name="pp", bufs=8))
        # PSUM: "sc" 2x[128,1024] (4 banks) shared by prep-transposes/proj/
        # qk-transposes/scores/scoresT/outproj; "d" 2x[128,1024] (4 banks)
        # for the dK then dQ accumulators (each head's accumulator owns a
        # whole tile so each has_written group has its own banks) -> 8 banks.
        ps_sc = ctx.enter_context(tc.tile_pool(name="ps_sc", bufs=2, space="PSUM"))
        ps_d = ctx.enter_context(tc.tile_pool(name="ps_d", bufs=2, space="PSUM"))

        ident = sb1.tile([128, 128], F32, tag="ident")
        make_identity(nc, ident[:])
        ident_r = sb1.tile([128, 128], F32R, tag="ident_r")
        nc.vector.tensor_copy(ident_r[:], ident[:])

        for _rep in range(reps):
            # ---- natural-layout input loads ---------------------------------
            wnq = sb1.tile([128, NPAIR * D], F32R, tag="wnq")  # [z2, (pair, d)]
            wnk = sb1.tile([128, NPAIR * D], F32R, tag="wnk")
            for p in range(NPAIR):
                nc.sync.dma_start(wnq[:, p * D : (p + 1) * D], w4q[p * 128 : (p + 1) * 128, :])
                nc.sync.dma_start(wnk[:, p * D : (p + 1) * D], w4k[p * 128 : (p + 1) * 128, :])

            # ---- on-device transposes: W^T [d, z2] and G^T [d, s] -----------
            wtq = sb1.tile([128, NPAIR * ND * 128], F32R, tag="wtq")  # [d, (pair,dt,z2)]
            wtk = sb1.tile([128, NPAIR * ND * 128], F32R, tag="wtk")
            for wn, wt in ((wnq, wtq), (wnk, wtk)):
                for p in range(NPAIR):
                    ps = ps_sc.tile([128, S], F32R, tag="ps_sc", name=f"wtp_{wt.tensor.name}{p}")
                    for dt in range(ND):
                        nc.tensor.transpose(
                            ps[:, dt * 128 : (dt + 1) * 128],
                            wn[:, p * D + dt * 128 : p * D + (dt + 1) * 128],
                            ident_r[:],
                        )
                    nc.vector.tensor_copy(
                        wt[:, p * ND * 128 : (p + 1) * ND * 128], ps[:]
                    )

            gt = sb1.tile([128, ND * S], F32R, tag="gt")  # G^T: [d_in_tile, (dt, s)]
            for i in range(NQ):
                gn = sb2.tile([128, D], F32R, tag="gn", name=f"gn{i}")
                nc.sync.dma_start(gn[:], xb[i * 128 : (i + 1) * 128, :])
                ps = ps_sc.tile([128, S], F32R, tag="ps_sc", name=f"gtp{i}")
                for dt in range(ND):
                    nc.tensor.transpose(
                        ps[:, dt * 128 : (dt + 1) * 128],
                        gn[:, dt * 128 : (dt + 1) * 128],
                        ident_r[:],
                    )
                for dt in range(ND):
                    nc.any.tensor_copy(
                        gt[:, dt * S + i * 128 : dt * S + (i + 1) * 128],
                        ps[:, dt * 128 : (dt + 1) * 128],
                    )

            # persistent across pairs
            dqt2 = sb1.tile([128, NPAIR * S], F32R, tag="dqt2")  # [z2, (pair, q)]
            dkt2 = sb1.tile([128, NPAIR * S], F32R, tag="dkt2")  # [z2, (pair, k)]
            zrowA = sb1.tile([1, S], F32, tag="zrowA")
            zrowB = sb1.tile([1, S], F32, tag="zrowB")
            ztsb = sb1.tile([16, 128], F32, tag="ztsb")

            pending_dq = []

            def emit_dq_burst():
                """dQ^T(unnorm) += K_i^T PT_i over all blocks, then Z-scale."""
                if not pending_dq:
                    return
                PT_a, k2n_a, zbc_ab, pa = pending_dq.pop()
                dq_ps = [
                    ps_d.tile([128, S], F32, tag="ps_d", name=f"dq_ps{pa}_{a}")
                    for a in range(2)
                ]
                for i in range(NQ):
                    for a in range(2):
                        for ch in range(NCH):
                            nc.tensor.matmul(
                                dq_ps[a][a * 64 : (a + 1) * 64, ch * 512 : (ch + 1) * 512],
                                lhsT=k2n_a[:, i * 128 + a * 64 : i * 128 + (a + 1) * 64],
                                rhs=PT_a[:, (a * NQ + i) * S + ch * 512 : (a * NQ + i) * S + ch * 512 + 512],
                                start=(i == 0),
                                stop=(i == NQ - 1),
                            )
                for a in range(2):
                    nc.vector.tensor_tensor(
                        dqt2[a * 64 : (a + 1) * 64, pa * S : (pa + 1) * S],
                        dq_ps[a][a * 64 : (a + 1) * 64, :],
                        zbc_ab[a][a * 64 : (a + 1) * 64, :],
                        MUL,
                    )

            for p in range(NPAIR):
                # ---- projections: QT2/KT2 [z2, s] ----------------------------
                qt2 = sb2.tile([128, S], F32R, tag="qt2")
                kt2 = sb2.tile([128, S], F32R, tag="kt2")
                for wt, dst in ((wtq, qt2), (wtk, kt2)):
                    ps = ps_sc.tile([128, S], F32, tag="ps_sc", name=f"pj{p}_{dst.tensor.name}")
                    for dt in range(ND):
                        j = p * ND + dt
                        for ch in range(NCH):
                            nc.tensor.matmul(
                                ps[:, ch * 512 : (ch + 1) * 512],
                                lhsT=wt[:, j * 128 : (j + 1) * 128],
                                rhs=gt[:, dt * S + ch * 512 : dt * S + ch * 512 + 512],
                                start=(dt == 0),
                                stop=(dt == ND - 1),
                            )
                    for ch in range(NCH):
                        nc.vector.tensor_copy(
                            dst[:, ch * 512 : (ch + 1) * 512],
                            ps[:, ch * 512 : (ch + 1) * 512],
                        )

                # ---- natural-layout transposes: Qraw / K2n [s, z2] -----------
                qraw = sb2.tile([128, S], F16, tag="qraw")
                k2n = sb2.tile([128, S], F16, tag="k2n")
                for src, dst in ((qt2, qraw), (kt2, k2n)):
                    ps = ps_sc.tile([128, S], F32R, tag="ps_sc", name=f"tr{p}_{dst.tensor.name}")
                    for i in range(NQ):
                        nc.tensor.transpose(
                            ps[:, i * 128 : (i + 1) * 128],
                            src[:, i * 128 : (i + 1) * 128],
                            ident_r[:],
                        )
                    nc.vector.tensor_copy(dst[:], ps[:])

                # previous pair's deferred dQ^T burst: emitted after this pair's
                # proj/transposes so the new scores/exps win scheduler priority
                emit_dq_burst()

                # ---- fused scores/exp/accumulate loop ------------------------
                zsum2 = sb2.tile([128, 16], F32, tag="zsum2")  # [(q), (head, qb)]
                dk_ps = [
                    ps_d.tile([128, S], F32, tag="ps_d", name=f"dk_ps{p}_{a}")
                    for a in range(2)
                ]
                PT_all = pp.tile([128, 2 * NQ * S], F16, tag="PT", bufs=1, name=f"PT{p}")
                for i in range(NQ):
                    # scores blocks [q_i, k] for both heads (row-tiled pairs)
                    pt_s = []
                    for a in range(2):
                        ps = ps_sc.tile([128, S], F32, tag="ps_sc", name=f"sc{p}_{i}_{a}")
                        for ch in range(NCH):
                            nc.tensor.matmul(
                                ps[:, ch * 512 : (ch + 1) * 512],
                                lhsT=qt2[a * 64 : (a + 1) * 64, i * 128 : (i + 1) * 128],
                                rhs=kt2[a * 64 : (a + 1) * 64, ch * 512 : (ch + 1) * 512],
                                start=True,
                                stop=True,
                            )
                        pt_s.append(ps)
                    # P blocks + Z row-sums
                    P_t = []
                    for a in range(2):
                        pb = pp.tile([128, S], F16, tag="P", name=f"P{p}_{i}_{a}")
                        nc.scalar.activation(
                            pb[:],
                            pt_s[a][:],
                            EXP,
                            scale=float(BETA),
                            accum_out=zsum2[:, a * NQ + i : a * NQ + i + 1],
                        )
                        P_t.append(pb)
                    # Qn block = Qraw_i / Z_i
                    q2n_t = sb4.tile([128, 128], F16, tag="q2n", name=f"q2n{p}_{i}")
                    for a in range(2):
                        zq = sb4.tile([128, 1], F32, tag="zq", name=f"zq{p}_{i}_{a}")
                        nc.vector.reciprocal(zq[:], zsum2[:, a * NQ + i : a * NQ + i + 1])
                        nc.vector.tensor_scalar_mul(
                            q2n_t[:, a * 64 : (a + 1) * 64],
                            qraw[:, i * 128 + a * 64 : i * 128 + (a + 1) * 64],
                            zq[:],
                        )
                    # dK^T += Qn_i^T P_i (col-tiled pair; each head's
                    # accumulator owns its own psum tile/banks)
                    for a in range(2):
                        for ch in range(NCH):
                            nc.tensor.matmul(
                                dk_ps[a][a * 64 : (a + 1) * 64, ch * 512 : (ch + 1) * 512],
                                lhsT=q2n_t[:, a * 64 : (a + 1) * 64],
                                rhs=P_t[a][:, ch * 512 : (ch + 1) * 512],
                                start=(i == 0),
                                stop=(i == NQ - 1),
                            )
                    # scoresT blocks [k_i, q] and PT
                    st_s = []
                    for a in range(2):
                        ps = ps_sc.tile([128, S], F32, tag="ps_sc", name=f"st{p}_{i}_{a}")
                        for ch in range(NCH):
                            nc.tensor.matmul(
                                ps[:, ch * 512 : (ch + 1) * 512],
                                lhsT=kt2[a * 64 : (a + 1) * 64, i * 128 : (i + 1) * 128],
                                rhs=qt2[a * 64 : (a + 1) * 64, ch * 512 : (ch + 1) * 512],
                                start=True,
                                stop=True,
                            )
                        st_s.append(ps)
                    for a in range(2):
                        j = a * NQ + i
                        nc.scalar.activation(
                            PT_all[:, j * S : (j + 1) * S], st_s[a][:], EXP, scale=float(BETA)
                        )

                # ---- Z^-1 broadcast [z2, q] then evacuate accumulators -------
                zinv2 = sb2.tile([128, 16], F32, tag="zinv2")
                nc.vector.reciprocal(zinv2[:], zsum2[:])
                zt_ps = ps_sc.tile([128, S], F32, tag="ps_sc", name=f"ztp{p}")
                nc.tensor.transpose(zt_ps[0:16, 0:128], zinv2[:], ident[:])
                nc.vector.tensor_copy(ztsb[:], zt_ps[0:16, 0:128])
                nc.sync.dma_start(
                    zrowA[:].rearrange("p (b c) -> p b c", b=NQ), ztsb[0:NQ, :]
                )
                nc.sync.dma_start(
                    zrowB[:].rearrange("p (b c) -> p b c", b=NQ), ztsb[NQ : 2 * NQ, :]
                )
                # partition_broadcast is only correct to base partition 0 ->
                # broadcast each head's Z row across a full tile, read halves.
                zbcA = sb2.tile([128, S], F32, tag="zbcA")
                zbcB = sb2.tile([128, S], F32, tag="zbcB")
                nc.gpsimd.partition_broadcast(zbcA[:], zrowA[:])
                nc.gpsimd.partition_broadcast(zbcB[:], zrowB[:])

                for a in range(2):
                    nc.vector.tensor_copy(
                        dkt2[a * 64 : (a + 1) * 64, p * S : (p + 1) * S],
                        dk_ps[a][a * 64 : (a + 1) * 64, :],
                    )

                # (the dQ^T burst for this pair is emitted lazily -- see
                # emit_dq_burst -- so the next pair's scores/exps get priority)
                pending_dq.append((PT_all, k2n, (zbcA, zbcB), p))

            emit_dq_burst()

            for tname, tt in (("wnq", wnq), ("wnk", wnk), ("wtq", wtq),
                              ("wtk", wtk), ("gt", gt), ("dqt2", dqt2),
                              ("dkt2", dkt2)):
                if tname in taps:
                    dshape = [tt.tensor.shape[0], tt.tensor.shape[1]]
                    dto = nc.dram_tensor(
                        f"tap_{tname}", dshape, tt.tensor.dtype, kind="ExternalOutput"
                    ).ap()
                    nc.sync.dma_start(dto[:, :], tt[:])

            # ---- output projection  -(sum_h dQ wq + dK wk) -------------------
            for sb in range(NQ):
                ps = ps_sc.tile([128, S], F32, tag="ps_sc", name=f"op{sb}")
                for ch in range(NCH):
                    n = 0
                    for p in range(NPAIR):
                        for dmat, wmat in ((dqt2, wnq), (dkt2, wnk)):
                            nc.tensor.matmul(
                                ps[:, ch * 512 : (ch + 1) * 512],
                                lhsT=dmat[:, p * S + sb * 128 : p * S + (sb + 1) * 128],
                                rhs=wmat[:, p * D + ch * 512 : p * D + ch * 512 + 512],
                                start=(n == 0),
                                stop=(n == 2 * NPAIR - 1),
                            )
                            n += 1
                go = sb4.tile([128, S], OUT_DT, tag="go", name=f"go{sb}")
                if out_mode.endswith("copy"):
                    nc.vector.tensor_copy(go[:], ps[:])
                else:
                    nc.scalar.activation(go[:], ps[:], COPY, scale=-1.0)
                nc.sync.dma_start(gout[sb * 128 : (sb + 1) * 128, :], go[:])

    nc.compile()
    return nc


def core_inputs(x, wq, wk, core):
    """Per-core input arrays -- natural layouts, pure views (no host work)."""
    b = core // 4
    h0 = 4 * (core % 4)
    return {
        "xb": x[b],
        "w4q": wq[h0 : h0 + 4].reshape(NPAIR * 128, D),
        "w4k": wk[h0 : h0 + 4].reshape(NPAIR * 128, D),
    }


def all_core_inputs(x, wq, wk):
    """in_maps for all cores; the x[b] view object is shared across the 4
    cores of batch b so the native path serializes it once per call."""
    xv = [x[b] for b in range(B)]
    maps = []
    for c in range(N_CORES):
        h0 = 4 * (c % 4)
        maps.append(
            {
                "xb": xv[c // 4],
                "w4q": wq[h0 : h0 + 4].reshape(NPAIR * 128, D),
                "w4k": wk[h0 : h0 + 4].reshape(NPAIR * 128, D),
            }
        )
    return maps


def combine(gouts):
    """Host unshard: negate+sum the per-batch positive partials, add pos.

    Memory-bandwidth bound, so split across threads (numpy releases the GIL
    for the large elementwise ops)."""
    pos = np.linspace(-0.5, 0.5, S, dtype=np.float32)[:, None] * np.float32(POS_SCALE)
    out = np.empty((B, S, D), np.float32)
    nchunk = 2
    cs = S // nchunk

    def one(b, j):
        sl = slice(j * cs, (j + 1) * cs)
        acc = gouts[4 * b][sl] + gouts[4 * b + 1][sl]
        np.add(acc, gouts[4 * b + 2][sl], out=acc)
        np.add(acc, gouts[4 * b + 3][sl], out=acc)
        np.subtract(pos[sl], acc, out=out[b][sl])

    ts = [
        threading.Thread(target=one, args=(b, j))
        for b in range(B)
        for j in range(nchunk)
    ]
    for t in ts:
        t.start()
    for t in ts:
        t.join()
    return out


def _out_specs(nc):
    """(name, shape, np-dtype) for each ExternalOutput of nc."""
    import concourse.mybir as mybir

    specs = []
    for alloc in nc.m.functions[0].allocations:
        if isinstance(alloc, mybir.MemoryLocationSet) and alloc.kind == "ExternalOutput":
            specs.append(
                (
                    alloc.memorylocations[0].name,
                    tuple(alloc.tensor_shape),
                    mybir.dt.np(alloc.dtype),
                )
            )
    return specs


def _native_make_sets(st, nc, core, sizes):
    """Persistent per-core NRT tensor sets (allocated once, reused per call)."""
    nrt = st["nrt"]
    ffi, lib = nrt.ffi, nrt.lib
    from concourse.libnrt import deref

    sets = {}
    for kind, names in (("in", st["in_names"]), ("out", [o[0] for o in st["outs"]])):
        set_ptr = ffi.new("nrt_tensor_set_t **")
        ret = lib.nrt_allocate_tensor_set(set_ptr)
        nrt.check_status(ret, "tensor set alloc failed")
        tensors = {}
        for name in names:
            t_ptr = ffi.new("nrt_tensor_t **")
            ret = lib.nrt_tensor_allocate(
                lib.NRT_TENSOR_PLACEMENT_DEVICE,
                core,
                sizes[name],
                name.encode(),
                t_ptr,
            )
            nrt.check_status(ret, f"tensor alloc {name} failed")
            ret = lib.nrt_add_tensor_to_tensor_set(
                deref(set_ptr), name.encode(), deref(t_ptr)
            )
            nrt.check_status(ret, f"add {name} to set failed")
            tensors[name] = (t_ptr, deref(t_ptr))
        sets[kind] = (set_ptr, tensors)
    return sets


def _run_native(nc, in_maps):
    """Fast native path: compile once, keep NEFF loaded and device tensor
    sets allocated across calls."""
    st = _CACHE.get("native")
    if st is None:
        import tempfile

        import concourse.mybir as mybir
        from concourse.bass_utils import compile_bass_kernel, initialize_nrt
        from concourse.libnrt import Krt

        tmpdir = tempfile.mkdtemp()
        neff_file = compile_bass_kernel(nc, tmpdir)
        nrt = initialize_nrt(has_collectives=nc.has_collectives)
        clients = []
        for c in range(N_CORES):
            k = Krt(nrt, core_id=c)
            k.load_model(
                neff_file, cc_enabled=nc.has_collectives, device_count=N_CORES
            )
            clients.append(k)
        part_name = (
            nc.partition_id_tensor.name if nc.partition_id_tensor else None
        )
        in_names, sizes = [], {}
        for alloc in nc.m.functions[0].allocations:
            if not isinstance(alloc, mybir.MemoryLocationSet):
                continue
            name = alloc.memorylocations[0].name
            nbytes = int(np.prod(alloc.tensor_shape)) * mybir.dt.size(alloc.dtype)
            if alloc.kind == "ExternalInput":
                in_names.append(name)
                sizes[name] = nbytes
            elif alloc.kind == "ExternalOutput":
                sizes[name] = nbytes
        st = {
            "nrt": nrt,
            "clients": clients,
            "outs": _out_specs(nc),
            "in_names": in_names,
            "part_name": part_name,
            "sizes": sizes,
        }
        try:
            st["sets"] = [
                _native_make_sets(st, nc, c, sizes) for c in range(N_CORES)
            ]
        except Exception:  # noqa: BLE001
            st["sets"] = None
        _CACHE["native"] = st

    part_name = st["part_name"]
    outs = [None] * N_CORES
    errs = []
    nrt = st["nrt"]

    # Contiguous per-core input arrays (views are already contiguous, so no
    # copies); the fast path writes them to the device with no host staging.
    arr_maps = []
    for c in range(N_CORES):
        m = {k2: np.ascontiguousarray(v) for k2, v in in_maps[c].items()}
        if part_name is not None:
            if "part_arrs" not in st:
                st["part_arrs"] = [
                    np.array([[i]], dtype=np.uint32) for i in range(N_CORES)
                ]
            m[part_name] = st["part_arrs"][c]
        arr_maps.append(m)

    def work_fast(c):
        from concourse.libnrt import deref

        lib, ffi = nrt.lib, nrt.ffi
        sets = st["sets"][c]
        in_set, in_tensors = sets["in"]
        out_set, out_tensors = sets["out"]
        for name, arr in arr_maps[c].items():
            t = in_tensors[name][1]
            ret = lib.nrt_tensor_write(t, ffi.from_buffer(arr), 0, arr.nbytes)
            nrt.check_status(ret, f"tensor write {name} failed")
        model = st["clients"][c].nrt_models[0]
        ret = lib.nrt_execute(model, deref(in_set), deref(out_set))
        nrt.check_status(ret, "nrt_execute failed")
        nm, shp, dt = st["outs"][0]
        out_arr = np.empty(shp, dt)
        ret = lib.nrt_tensor_read(
            out_tensors[nm][1],
            ffi.from_buffer(out_arr, require_writable=True),
            0,
            out_arr.nbytes,
        )
        nrt.check_status(ret, f"tensor read {nm} failed")
        outs[c] = out_arr

    def work(c):
        try:
            if st["sets"] is not None:
                try:
                    work_fast(c)
                    return
                except Exception:  # noqa: BLE001
                    pass
            ser_map = {k2: v.tobytes() for k2, v in arr_maps[c].items()}
            outputs_c = {
                nm: np.zeros(shp, dt).tobytes() for nm, shp, dt in st["outs"]
            }
            st["clients"][c].model_execute(0, ser_map, outputs_c)
            nm, shp, dt = st["outs"][0]
            outs[c] = np.frombuffer(outputs_c[nm], dt).reshape(shp)
        except Exception as e:  # noqa: BLE001
            errs.append(e)

    threads = [threading.Thread(target=work, args=(c,)) for c in range(N_CORES)]
    for t in threads:
        t.start()
    for t in threads:
        t.join()
    if errs:
        raise errs[0]
    return outs


def _make_axon_callable(nc):
    """Persistent jit'd sharded runner for the axon (tunneled-PJRT) path."""
    import jax
    from jax.experimental.shard_map import shard_map
    from jax.sharding import Mesh, NamedSharding, PartitionSpec

    import concourse.mybir as mybir
    from concourse import bass2jax

    bass2jax.install_neuronx_cc_hook()
    partition_name = nc.partition_id_tensor.name if nc.partition_id_tensor else None
    in_names, out_names, out_avals, zero_outs = [], [], [], []
    for alloc in nc.m.functions[0].allocations:
        if not isinstance(alloc, mybir.MemoryLocationSet):
            continue
        name = alloc.memorylocations[0].name
        if alloc.kind == "ExternalInput":
            if name != partition_name:
                in_names.append(name)
        elif alloc.kind == "ExternalOutput":
            shape = tuple(alloc.tensor_shape)
            dtype = mybir.dt.np(alloc.dtype)
            out_names.append(name)
            out_avals.append(jax.core.ShapedArray(shape, dtype))
            zero_outs.append(np.zeros(shape, dtype))
    n_params = len(in_names)
    all_in_names = list(in_names) + out_names
    if partition_name is not None:
        all_in_names.append(partition_name)

    def _body(*args):
        operands = list(args)
        if partition_name is not None:
            operands.append(bass2jax.partition_id_tensor())
        outs = bass2jax._bass_exec_p.bind(
            *operands,
            out_avals=tuple(out_avals),
            in_names=tuple(all_in_names),
            out_names=tuple(out_names),
            lowering_input_output_aliases=(),
            sim_require_finite=True,
            sim_require_nnan=True,
            nc=nc,
        )
        return tuple(outs)

    devices = jax.devices()[:N_CORES]
    mesh = Mesh(np.asarray(devices), ("core",))
    spec = PartitionSpec("core")
    sharded = jax.jit(
        shard_map(
            _body,
            mesh=mesh,
            in_specs=(spec,) * (n_params + len(out_names)),
            out_specs=(spec,) * len(out_names),
            check_rep=False,
        ),
        donate_argnums=tuple(range(n_params, n_params + len(out_names))),
        keep_unused=True,
    )
    sh = NamedSharding(mesh, spec)
    state = {"prev": None}

    def call(in_maps):
        concat_in = [
            jax.device_put(
                np.concatenate([np.asarray(m[nm]) for m in in_maps], axis=0), sh
            )
            for nm in in_names
        ]
        if state["prev"] is None:
            donated = [
                jax.device_put(
                    np.zeros((N_CORES * z.shape[0],) + z.shape[1:], z.dtype), sh
                )
                for z in zero_outs
            ]
        else:
            # gout is fully written by the NEFF, so last call's outputs are
            # valid donation fodder -- avoids re-uploading zeros every call.
            donated = state["prev"]
        state["prev"] = None  # consumed by donation even if the call fails
        out_arrs = sharded(*concat_in, *donated)
        state["prev"] = list(out_arrs)
        host = [np.asarray(a) for a in out_arrs]
        return [
            {nm: host[i].reshape(N_CORES, *out_avals[i].shape)[c]
             for i, nm in enumerate(out_names)}
            for c in range(N_CORES)
        ]

    return call


def _run(nc, in_maps):
    from concourse.bass_utils import axon_active

    if axon_active():
        try:
            if "axon_call" not in _CACHE:
                _CACHE["axon_call"] = _make_axon_callable(nc)
            res = _CACHE["axon_call"](in_maps)
            return [r["gout"] for r in res]
        except Exception:  # noqa: BLE001 - fall through to stock path
            pass
    else:
        try:
            return _run_native(nc, in_maps)
        except Exception:  # noqa: BLE001 - fall through to stock path
            pass
    from concourse.bass_utils import run_bass_kernel_spmd

    res = run_bass_kernel_spmd(nc, in_maps, core_ids=list(range(N_CORES)))
    return [r["gout"] for r in res.results]


def kernel(x, wq, wk, trace=False):
    x = np.asarray(x, np.float32)
    wq = np.asarray(wq, np.float32)
    wk = np.asarray(wk, np.float32)
    if "nc" not in _CACHE:
        _CACHE["nc"] = build_nc()
    nc = _CACHE["nc"]

    in_maps = all_core_inputs(x, wq, wk)
    if trace:
        from concourse.bass_utils import run_bass_kernel_spmd

        res = run_bass_kernel_spmd(
            nc, in_maps, core_ids=list(range(N_CORES)), trace=True
        )
        _CACHE["last_result"] = res
        gouts = [r["gout"] for r in res.results]
    else:
        gouts = _run(nc, in_maps)
    return combine(gouts)


# revision 33
# speedup vs baseline: 19.9897x; 19.9897x over previous
"""Trainium2 Bass kernel: gradient of the EnergyAttention scalar energy.

reference:
    q = einsum('bqd,hzd->bqhz', g, wq); k = einsum('bkd,hzd->bkhz', g, wk)
    scores = einsum('bqhz,bkhz->bhqk', q, k)
    E = -(logsumexp(BETA*scores, -1)/BETA).sum() + POS_SCALE*(g*pos).sum()
    out = dE/dg

Math: with P = softmax(BETA*scores) per (b,h,q):
    out[b] = -sum_h [ (P@K) @ wq_h + (P.T@Qn) @ wk_h ] + POS_SCALE*pos
where Qn = diag(1/Z) Q (row-normalized by the softmax partition Z).

Sharding: 8 cores; core c handles batch b=c//4 and heads 4*(c%4)..4*(c%4)+3
(two head-pairs packed into the 128-partition dim).  Each core takes its
inputs in NATURAL layout (x[b], wq[h0:h0+4], wk[h0:h0+4] -- pure views, no
host-side packing) and emits the positive partial sum_h[(dQ)wq + (dK)wk]
in f32; the host negates, sums the four per-core partials of each batch,
and adds the positional term.

Per-core structure (fused pipeline; P tiles are consumed by the transposed
projection matmuls as soon as they are exp'd, so no full [S,S] matrix is
ever materialized):
  prep:   G^T, wq^T, wk^T built on-device via PE transpose-mode
  proj:   QT2/KT2 [z2, s] = (W GT) via f32r matmuls (d contracted in 8 tiles)
  trans:  Qraw/K2n [s, z2] via PE transpose-mode
  loop i: scores/scoresT blocks (row-tiled K=64 pairs, f32r)
          -> exp on ACT (fp16 out, fused row-sum accum for Z)
          -> dK^T += (Q/Z)^T-block @ P-block, dQ^T += K-block @ PT-block
             (col-tiled M=64 pairs, fp16, single has_written group per bank)
  out:    partial = sum_pairs dQT^T wq + dKT^T wk  (K=128-packed f32r)
"""

import threading

import numpy as np

B = 2
S = 1024
D = 1024
NH = 16
Z = 64
BETA = 1.0 / np.sqrt(np.float32(Z))
POS_SCALE = 0.001
N_CORES = 8
HPC = 4           # heads per core
NPAIR = 2         # head pairs per core
ND = D // 128     # 8 d-tiles
NQ = S // 128     # 8 q/k blocks
NCH = S // 512    # 2 moving-dim chunks

_CACHE = {}


def build_nc(reps=1, taps=(), out_mode="f32copy"):
    """Build the (SPMD, identical-per-core) Bass program.

    reps>1 repeats the whole computation (idempotent) inside one NEFF --
    used to measure steady-state per-execution time as a marginal cost.
    taps: debug-only -- names of internal tiles to DMA out as extra outputs.
    out_mode: "f32copy" (default -- positive partial, host negates; the
    baseline-proven evacuation).  fp16 external outputs are corrupted by a
    f32->f16 conversion misapplied to packed f16 pairs somewhere in the
    output plumbing (odd columns bit-exact, even columns zeroed/exponent-
    shifted), so "f16neg"/"f16copy" are debug-only."""
    from contextlib import ExitStack

    import concourse.mybir as mybir
    import concourse.tile as tile
    from concourse import bacc
    from concourse.masks import make_identity

    F32 = mybir.dt.float32
    F32R = mybir.dt.float32r
    F16 = mybir.dt.float16
    MUL = mybir.AluOpType.mult
    EXP = mybir.ActivationFunctionType.Exp
    COPY = mybir.ActivationFunctionType.Copy

    nc = bacc.Bacc(
        "TRN2",
        target_bir_lowering=False,
        debug=False,
        enable_asserts=False,
        num_devices=N_CORES,
    )

    OUT_DT = F32 if out_mode.startswith("f32") else F16
    xb = nc.dram_tensor("xb", [S, D], F32R, kind="ExternalInput").ap()
    w4q = nc.dram_tensor("w4q", [NPAIR * 128, D], F32R, kind="ExternalInput").ap()
    w4k = nc.dram_tensor("w4k", [NPAIR * 128, D], F32R, kind="ExternalInput").ap()
    gout = nc.dram_tensor("gout", [S, D], OUT_DT, kind="ExternalOutput").ap()

    with tile.TileContext(nc) as tc, ExitStack() as ctx:
        sb1 = ctx.enter_context(tc.tile_pool(name="sb1", bufs=1))
        sb2 = ctx.enter_context(tc.tile_pool(name="sb2", bufs=2))
        sb4 = ctx.enter_context(tc.tile_pool(name="sb4", bufs=4))
        pp = ctx.enter_context(tc.tile_pool(name="pp", bufs=8))
        # PSUM: "sc" 2x[128,1024] (4 banks) shared by prep-transposes/proj/
        # qk-transposes/scores/scoresT/outproj; "d" 2x[128,1024] (4 banks)
        # for the dK then dQ accumulators (each head's accumulator owns a
        # whole tile so each has_written group has its own banks) -> 8 banks.
        ps_sc = ctx.enter_context(tc.tile_pool(name="ps_sc", bufs=2, space="PSUM"))
        ps_d = ctx.enter_context(tc.tile_pool(name="ps_d", bufs=2, space="PSUM"))

        ident = sb1.tile([128, 128], F32, tag="ident")
        make_identity(nc, ident[:])
        ident_r = sb1.tile([128, 128], F32R, tag="ident_r")
        nc.vector.tensor_copy(ident_r[:], ident[:])

        for _rep in range(reps):
            # ---- natural-layout input loads ---------------------------------
            wnq = sb1.tile([128, NPAIR * D], F32R, tag="wnq")  # [z2, (pair, d)]
            wnk = sb1.tile([128, NPAIR * D], F32R, tag="wnk")
            for p in range(NPAIR):
                nc.sync.dma_start(wnq[:, p * D : (p + 1) * D], w4q[p * 128 : (p + 1) * 128, :])
                nc.sync.dma_start(wnk[:, p * D : (p + 1) * D], w4k[p * 128 : (p + 1) * 128, :])

            # ---- on-device transposes: W^T [d, z2] and G^T [d, s] -----------
            wtq = sb1.tile([128, NPAIR * ND * 128], F32R, tag="wtq")  # [d, (pair,dt,z2)]
            wtk = sb1.tile([128, NPAIR * ND * 128], F32R, tag="wtk")
            for wn, wt in ((wnq, wtq), (wnk, wtk)):
                for p in range(NPAIR):
                    ps = ps_sc.tile([128, S], F32R, tag="ps_sc", name=f"wtp_{wt.tensor.name}{p}")
                    for dt in range(ND):
                        nc.tensor.transpose(
                            ps[:, dt * 128 : (dt + 1) * 128],
                            wn[:, p * D + dt * 128 : p * D + (dt + 1) * 128],
                            ident_r[:],
                        )
                    nc.vector.tensor_copy(
                        wt[:, p * ND * 128 : (p + 1) * ND * 128], ps[:]
                    )

            gt = sb1.tile([128, ND * S], F32R, tag="gt")  # G^T: [d_in_tile, (dt, s)]
            for i in range(NQ):
                gn = sb2.tile([128, D], F32R, tag="gn", name=f"gn{i}")
                nc.sync.dma_start(gn[:], xb[i * 128 : (i + 1) * 128, :])
                ps = ps_sc.tile([128, S], F32R, tag="ps_sc", name=f"gtp{i}")
                for dt in range(ND):
                    nc.tensor.transpose(
                        ps[:, dt * 128 : (dt + 1) * 128],
                        gn[:, dt * 128 : (dt + 1) * 128],
                        ident_r[:],
                    )
                for dt in range(ND):
                    nc.any.tensor_copy(
                        gt[:, dt * S + i * 128 : dt * S + (i + 1) * 128],
                        ps[:, dt * 128 : (dt + 1) * 128],
                    )

            # persistent across pairs
            dqt2 = sb1.tile([128, NPAIR * S], F32R, tag="dqt2")  # [z2, (pair, q)]
            dkt2 = sb1.tile([128, NPAIR * S], F32R, tag="dkt2")  # [z2, (pair, k)]
            zrowA = sb1.tile([1, S], F32, tag="zrowA")
            zrowB = sb1.tile([1, S], F32, tag="zrowB")
            ztsb = sb1.tile([16, 128], F32, tag="ztsb")

            pending_dq = []

            def emit_dq_burst():
                """dQ^T(unnorm) += K_i^T PT_i over all blocks, then Z-scale."""
                if not pending_dq:
                    return
                PT_a, k2n_a, zbc_ab, pa = pending_dq.pop()
                dq_ps = [
                    ps_d.tile([128, S], F32, tag="ps_d", name=f"dq_ps{pa}_{a}")
                    for a in range(2)
                ]
                for i in range(NQ):
                    for a in range(2):
                        for ch in range(NCH):
                            nc.tensor.matmul(
                                dq_ps[a][a * 64 : (a + 1) * 64, ch * 512 : (ch + 1) * 512],
                                lhsT=k2n_a[:, i * 128 + a * 64 : i * 128 + (a + 1) * 64],
                                rhs=PT_a[:, (a * NQ + i) * S + ch * 512 : (a * NQ + i) * S + ch * 512 + 512],
                                start=(i == 0),
                                stop=(i == NQ - 1),
                            )
                for a in range(2):
                    nc.vector.tensor_tensor(
                        dqt2[a * 64 : (a + 1) * 64, pa * S : (pa + 1) * S],
                        dq_ps[a][a * 64 : (a + 1) * 64, :],
                        zbc_ab[a][a * 64 : (a + 1) * 64, :],
                        MUL,
                    )

            for p in range(NPAIR):
                # ---- projections: QT2/KT2 [z2, s] ----------------------------
                qt2 = sb2.tile([128, S], F32R, tag="qt2")
                kt2 = sb2.tile([128, S], F32R, tag="kt2")
                for wt, dst in ((wtq, qt2), (wtk, kt2)):
                    ps = ps_sc.tile([128, S], F32, tag="ps_sc", name=f"pj{p}_{dst.tensor.name}")
                    for dt in range(ND):
                        j = p * ND + dt
                        for ch in range(NCH):
                            nc.tensor.matmul(
                                ps[:, ch * 512 : (ch + 1) * 512],
                                lhsT=wt[:, j * 128 : (j + 1) * 128],
                                rhs=gt[:, dt * S + ch * 512 : dt * S + ch * 512 + 512],
                                start=(dt == 0),
                                stop=(dt == ND - 1),
                            )
                    for ch in range(NCH):
                        nc.vector.tensor_copy(
                            dst[:, ch * 512 : (ch + 1) * 512],
                            ps[:, ch * 512 : (ch + 1) * 512],
                        )

                # ---- natural-layout transposes: Qraw / K2n [s, z2] -----------
                qraw = sb2.tile([128, S], F16, tag="qraw")
                k2n = sb2.tile([128, S], F16, tag="k2n")
                for src, dst in ((qt2, qraw), (kt2, k2n)):
                    ps = ps_sc.tile([128, S], F32R, tag="ps_sc", name=f"tr{p}_{dst.tensor.name}")
                    for i in range(NQ):
                        nc.tensor.transpose(
                            ps[:, i * 128 : (i + 1) * 128],
                            src[:, i * 128 : (i + 1) * 128],
                            ident_r[:],
                        )
                    nc.vector.tensor_copy(dst[:], ps[:])

                # previous pair's deferred dQ^T burst: emitted after this pair's
                # proj/transposes so the new scores/exps win scheduler priority
                emit_dq_burst()

                # ---- fused scores/exp/accumulate loop ------------------------
                zsum2 = sb2.tile([128, 16], F32, tag="zsum2")  # [(q), (head, qb)]
                dk_ps = [
                    ps_d.tile([128, S], F32, tag="ps_d", name=f"dk_ps{p}_{a}")
                    for a in range(2)
                ]
                PT_all = pp.tile([128, 2 * NQ * S], F16, tag="PT", bufs=1, name=f"PT{p}")
                for i in range(NQ):
                    # scores blocks [q_i, k] for both heads (row-tiled pairs)
                    pt_s = []
                    for a in range(2):
                        ps = ps_sc.tile([128, S], F32, tag="ps_sc", name=f"sc{p}_{i}_{a}")
                        for ch in range(NCH):
                            nc.tensor.matmul(
                                ps[:, ch * 512 : (ch + 1) * 512],
                                lhsT=qt2[a * 64 : (a + 1) * 64, i * 128 : (i + 1) * 128],
                                rhs=kt2[a * 64 : (a + 1) * 64, ch * 512 : (ch + 1) * 512],
                                start=True,
                                stop=True,
                            )
                        pt_s.append(ps)
                    # P blocks + Z row-sums
                    P_t = []
                    for a in range(2):
                        pb = pp.tile([128, S], F16, tag="P", name=f"P{p}_{i}_{a}")
                        nc.scalar.activation(
                            pb[:],
                            pt_s[a][:],
                            EXP,
                            scale=float(BETA),
                            accum_out=zsum2[:, a * NQ + i : a * NQ + i + 1],
                        )
                        P_t.append(pb)
                    # Qn block = Qraw_i / Z_i
                    q2n_t = sb4.tile([128, 128], F16, tag="q2n", name=f"q2n{p}_{i}")
                    for a in range(2):
                        zq = sb4.tile([128, 1], F32, tag="zq", name=f"zq{p}_{i}_{a}")
                        nc.vector.reciprocal(zq[:], zsum2[:, a * NQ + i : a * NQ + i + 1])
                        nc.vector.tensor_scalar_mul(
                            q2n_t[:, a * 64 : (a + 1) * 64],
                            qraw[:, i * 128 + a * 64 : i * 128 + (a + 1) * 64],
                            zq[:],
                        )
                    # dK^T += Qn_i^T P_i (col-tiled pair; each head's
                    # accumulator owns its own psum tile/banks)
                    for a in range(2):
                        for ch in range(NCH):
                            nc.tensor.matmul(
                                dk_ps[a][a * 64 : (a + 1) * 64, ch * 512 : (ch + 1) * 512],
                                lhsT=q2n_t[:, a * 64 : (a + 1) * 64],
                                rhs=P_t[a][:, ch * 512 : (ch + 1) * 512],
                                start=(i == 0),
                                stop=(i == NQ - 1),
                            )
                    # scoresT blocks [k_i, q] and PT
                    st_s = []
                    for a in range(2):
                        ps = ps_sc.tile([128, S], F32, tag="ps_sc", name=f"st{p}_{i}_{a}")
                        for ch in range(NCH):
                            nc.tensor.matmul(
                                ps[:, ch * 512 : (ch + 1) * 512],
                                lhsT=kt2[a * 64 : (a + 1) * 64, i * 128 : (i + 1) * 128],
                                rhs=qt2[a * 64 : (a + 1) * 64, ch * 512 : (ch + 1) * 512],
                                start=True,
                                stop=True,
                            )
                        st_s.append(ps)
                    for a in range(2):
                        j = a * NQ + i
                        nc.scalar.activation(
                            PT_all[:, j * S : (j + 1) * S], st_s[a][:], EXP, scale=float(BETA)
                        )

                # ---- Z^-1 broadcast [z2, q] then evacuate accumulators -------
                zinv2 = sb2.tile([128, 16], F32, tag="zinv2")
                nc.vector.reciprocal(zinv2[:], zsum2[:])
                zt_ps = ps_sc.tile([128, S], F32, tag="ps_sc", name=f"ztp{p}")
                nc.tensor.transpose(zt_ps[0:16, 0:128], zinv2[:], ident[:])
                nc.vector.tensor_copy(ztsb[:], zt_ps[0:16, 0:128])
                nc.sync.dma_start(
                    zrowA[:].rearrange("p (b c) -> p b c", b=NQ), ztsb[0:NQ, :]
                )
                nc.sync.dma_start(
                    zrowB[:].rearrange("p (b c) -> p b c", b=NQ), ztsb[NQ : 2 * NQ, :]
                )
                # partition_broadcast is only correct to base partition 0 ->
                # broadcast each head's Z row across a full tile, read halves.
                zbcA = sb2.tile([128, S], F32, tag="zbcA")
                zbcB = sb2.tile([128, S], F32, tag="zbcB")
                nc.gpsimd.partition_broadcast(zbcA[:], zrowA[:])
                nc.gpsimd.partition_broadcast(zbcB[:], zrowB[:])

                for a in range(2):
                    nc.vector.tensor_copy(
                        dkt2[a * 64 : (a + 1) * 64, p * S : (p + 1) * S],
                        dk_ps[a][a * 64 : (a + 1) * 64, :],
                    )

                # (the dQ^T burst for this pair is emitted lazily -- see
                # emit_dq_burst -- so the next pair's scores/exps get priority)
                pending_dq.append((PT_all, k2n, (zbcA, zbcB), p))

            emit_dq_burst()

            for tname, tt in (("wnq", wnq), ("wnk", wnk), ("wtq", wtq),
                              ("wtk", wtk), ("gt", gt), ("dqt2", dqt2),
                              ("dkt2", dkt2)):
                if tname in taps:
                    dshape = [tt.tensor.shape[0], tt.tensor.shape[1]]
                    dto = nc.dram_tensor(
                        f"tap_{tname}", dshape, tt.tensor.dtype, kind="ExternalOutput"
                    ).ap()
                    nc.sync.dma_start(dto[:, :], tt[:])

            # ---- output projection  -(sum_h dQ wq + dK wk) -------------------
            for sb in range(NQ):
                ps = ps_sc.tile([128, S], F32, tag="ps_sc", name=f"op{sb}")
                for ch in range(NCH):
                    n = 0
                    for p in range(NPAIR):
                        for dmat, wmat in ((dqt2, wnq), (dkt2, wnk)):
                            nc.tensor.matmul(
                                ps[:, ch * 512 : (ch + 1) * 512],
                                lhsT=dmat[:, p * S + sb * 128 : p * S + (sb + 1) * 128],
                                rhs=wmat[:, p * D + ch * 512 : p * D + ch * 512 + 512],
                                start=(n == 0),
                                stop=(n == 2 * NPAIR - 1),
                            )
                            n += 1
                go = sb4.tile([128, S], OUT_DT, tag="go", name=f"go{sb}")
                if out_mode.endswith("copy"):
                    nc.vector.tensor_copy(go[:], ps[:])
                else:
                    nc.scalar.activation(go[:], ps[:], COPY, scale=-1.0)
                nc.sync.dma_start(gout[sb * 128 : (sb + 1) * 128, :], go[:])

    nc.compile()
    return nc


def core_inputs(x, wq, wk, core):
    """Per-core input arrays -- natural layouts, pure views (no host work)."""
    b = core // 4
    h0 = 4 * (core % 4)
    return {
        "xb": x[b],
        "w4q": wq[h0 : h0 + 4].reshape(NPAIR * 128, D),
        "w4k": wk[h0 : h0 + 4].reshape(NPAIR * 128, D),
    }


def all_core_inputs(x, wq, wk):
    """in_maps for all cores; the x[b] view object is shared across the 4
    cores of batch b so the native path serializes it once per call."""
    xv = [x[b] for b in range(B)]
    maps = []
    for c in range(N_CORES):
        h0 = 4 * (c % 4)
        maps.append(
            {
                "xb": xv[c // 4],
                "w4q": wq[h0 : h0 + 4].reshape(NPAIR * 128, D),
                "w4k": wk[h0 : h0 + 4].reshape(NPAIR * 128, D),
            }
        )
    return maps


def combine(gouts):
    """Host unshard: negate+sum the per-batch positive partials, add pos.

    Memory-bandwidth bound, so split across threads (numpy releases the GIL
    for the large elementwise ops)."""
    pos = np.linspace(-0.5, 0.5, S, dtype=np.float32)[:, None] * np.float32(POS_SCALE)
    out = np.empty((B, S, D), np.float32)
    nchunk = 2
    cs = S // nchunk

    def one(b, j):
        sl = slice(j * cs, (j + 1) * cs)
        acc = gouts[4 * b][sl] + gouts[4 * b + 1][sl]
        np.add(acc, gouts[4 * b + 2][sl], out=acc)
        np.add(acc, gouts[4 * b + 3][sl], out=acc)
        np.subtract(pos[sl], acc, out=out[b][sl])

    ts = [
        threading.Thread(target=one, args=(b, j))
        for b in range(B)
        for j in range(nchunk)
    ]
    for t in ts:
        t.start()
    for t in ts:
        t.join()
    return out


def _out_specs(nc):
    """(name, shape, np-dtype) for each ExternalOutput of nc."""
    import concourse.mybir as mybir

    specs = []
    for alloc in nc.m.functions[0].allocations:
        if isinstance(alloc, mybir.MemoryLocationSet) and alloc.kind == "ExternalOutput":
            specs.append(
                (
                    alloc.memorylocations[0].name,
                    tuple(alloc.tensor_shape),
                    mybir.dt.np(alloc.dtype),
                )
            )
    return specs


def _native_make_sets(st, nc, core, sizes):
    """Persistent per-core NRT tensor sets (allocated once, reused per call)."""
    nrt = st["nrt"]
    ffi, lib = nrt.ffi, nrt.lib
    from concourse.libnrt import deref

    sets = {}
    for kind, names in (("in", st["in_names"]), ("out", [o[0] for o in st["outs"]])):
        set_ptr = ffi.new("nrt_tensor_set_t **")
        ret = lib.nrt_allocate_tensor_set(set_ptr)
        nrt.check_status(ret, "tensor set alloc failed")
        tensors = {}
        for name in names:
            t_ptr = ffi.new("nrt_tensor_t **")
            ret = lib.nrt_tensor_allocate(
                lib.NRT_TENSOR_PLACEMENT_DEVICE,
                core,
                sizes[name],
                name.encode(),
                t_ptr,
            )
            nrt.check_status(ret, f"tensor alloc {name} failed")
            ret = lib.nrt_add_tensor_to_tensor_set(
                deref(set_ptr), name.encode(), deref(t_ptr)
            )
            nrt.check_status(ret, f"add {name} to set failed")
            tensors[name] = (t_ptr, deref(t_ptr))
        sets[kind] = (set_ptr, tensors)
    return sets


def _run_native(nc, in_maps):
    """Fast native path: compile once, keep NEFF loaded and device tensor
    sets allocated across calls."""
    st = _CACHE.get("native")
    if st is None:
        import tempfile

        import concourse.mybir as mybir
        from concourse.bass_utils import compile_bass_kernel, initialize_nrt
        from concourse.libnrt import Krt

        tmpdir = tempfile.mkdtemp()
        neff_file = compile_bass_kernel(nc, tmpdir)
        nrt = initialize_nrt(has_collectives=nc.has_collectives)
        clients = []
        for c in range(N_CORES):
            k = Krt(nrt, core_id=c)
            k.load_model(
                neff_file, cc_enabled=nc.has_collectives, device_count=N_CORES
            )
            clients.append(k)
        part_name = (
            nc.partition_id_tensor.name if nc.partition_id_tensor else None
        )
        in_names, sizes = [], {}
        for alloc in nc.m.functions[0].allocations:
            if not isinstance(alloc, mybir.MemoryLocationSet):
                continue
            name = alloc.memorylocations[0].name
            nbytes = int(np.prod(alloc.tensor_shape)) * mybir.dt.size(alloc.dtype)
            if alloc.kind == "ExternalInput":
                in_names.append(name)
                sizes[name] = nbytes
            elif alloc.kind == "ExternalOutput":
                sizes[name] = nbytes
        st = {
            "nrt": nrt,
            "clients": clients,
            "outs": _out_specs(nc),
            "in_names": in_names,
            "part_name": part_name,
            "sizes": sizes,
        }
        try:
            st["sets"] = [
                _native_make_sets(st, nc, c, sizes) for c in range(N_CORES)
            ]
        except Exception:  # noqa: BLE001
            st["sets"] = None
        _CACHE["native"] = st

    part_name = st["part_name"]
    outs = [None] * N_CORES
    errs = []
    nrt = st["nrt"]

    # Contiguous per-core input arrays (views are already contiguous, so no
    # copies); the fast path writes them to the device with no host staging.
    arr_maps = []
    for c in range(N_CORES):
        m = {k2: np.ascontiguousarray(v) for k2, v in in_maps[c].items()}
        if part_name is not None:
            if "part_arrs" not in st:
                st["part_arrs"] = [
                    np.array([[i]], dtype=np.uint32) for i in range(N_CORES)
                ]
            m[part_name] = st["part_arrs"][c]
        arr_maps.append(m)

    def work_fast(c):
        from concourse.libnrt import deref

        lib, ffi = nrt.lib, nrt.ffi
        sets = st["sets"][c]
        in_set, in_tensors = sets["in"]
        out_set, out_tensors = sets["out"]
        for name, arr in arr_maps[c].items():
            t = in_tensors[name][1]
            ret = lib.nrt_tensor_write(t, ffi.from_buffer(arr), 0, arr.nbytes)
            nrt.check_status(ret, f"tensor write {name} failed")
        model = st["clients"][c].nrt_models[0]
        ret = lib.nrt_execute(model, deref(in_set), deref(out_set))
        nrt.check_status(ret, "nrt_execute failed")
        nm, shp, dt = st["outs"][0]
        out_arr = np.empty(shp, dt)
        ret = lib.nrt_tensor_read(
            out_tensors[nm][1],
            ffi.from_buffer(out_arr, require_writable=True),
            0,
            out_arr.nbytes,
        )
        nrt.check_status(ret, f"tensor read {nm} failed")
        outs[c] = out_arr

    def work(c):
        try:
            if st["sets"] is not None:
                try:
                    work_fast(c)
                    return
                except Exception:  # noqa: BLE001
                    pass
            ser_map = {k2: v.tobytes() for k2, v in arr_maps[c].items()}
            outputs_c = {
                nm: np.zeros(shp, dt).tobytes() for nm, shp, dt in st["outs"]
            }
            st["clients"][c].model_execute(0, ser_map, outputs_c)
            nm, shp, dt = st["outs"][0]
            outs[c] = np.frombuffer(outputs_c[nm], dt).reshape(shp)
        except Exception as e:  # noqa: BLE001
            errs.append(e)

    threads = [threading.Thread(target=work, args=(c,)) for c in range(N_CORES)]
    for t in threads:
        t.start()
    for t in threads:
        t.join()
    if errs:
        raise errs[0]
    return outs


def _make_axon_callable(nc):
    """Persistent jit'd sharded runner for the axon (tunneled-PJRT) path."""
    import jax

    try:
        from jax.experimental.shard_map import shard_map as _smap

        def shard_map(f, mesh, in_specs, out_specs):
            return _smap(
                f, mesh=mesh, in_specs=in_specs, out_specs=out_specs,
                check_rep=False,
            )
    except ImportError:  # jax >= 0.9: experimental alias removed

        def shard_map(f, mesh, in_specs, out_specs):
            return jax.shard_map(
                f, mesh=mesh, in_specs=in_specs, out_specs=out_specs,
                check_vma=False,
            )

    from jax.sharding import Mesh, NamedSharding, PartitionSpec

    import concourse.mybir as mybir
    from concourse import bass2jax

    bass2jax.install_neuronx_cc_hook()
    partition_name = nc.partition_id_tensor.name if nc.partition_id_tensor else None
    in_names, out_names, out_avals, zero_outs = [], [], [], []
    for alloc in nc.m.functions[0].allocations:
        if not isinstance(alloc, mybir.MemoryLocationSet):
            continue
        name = alloc.memorylocations[0].name
        if alloc.kind == "ExternalInput":
            if name != partition_name:
                in_names.append(name)
        elif alloc.kind == "ExternalOutput":
            shape = tuple(alloc.tensor_shape)
            dtype = mybir.dt.np(alloc.dtype)
            out_names.append(name)
            out_avals.append(jax.core.ShapedArray(shape, dtype))
            zero_outs.append(np.zeros(shape, dtype))
    n_params = len(in_names)
    all_in_names = list(in_names) + out_names
    if partition_name is not None:
        all_in_names.append(partition_name)

    def _body(*args):
        operands = list(args)
        if partition_name is not None:
            operands.append(bass2jax.partition_id_tensor())
        outs = bass2jax._bass_exec_p.bind(
            *operands,
            out_avals=tuple(out_avals),
            in_names=tuple(all_in_names),
            out_names=tuple(out_names),
            lowering_input_output_aliases=(),
            sim_require_finite=True,
            sim_require_nnan=True,
            nc=nc,
        )
        return tuple(outs)

    devices = jax.devices()[:N_CORES]
    mesh = Mesh(np.asarray(devices), ("core",))
    spec = PartitionSpec("core")
    sharded = jax.jit(
        shard_map(
            _body,
            mesh,
            (spec,) * (n_params + len(out_names)),
            (spec,) * len(out_names),
        ),
        donate_argnums=tuple(range(n_params, n_params + len(out_names))),
        keep_unused=True,
    )
    sh = NamedSharding(mesh, spec)
    state = {"prev": None}

    def call(in_maps):
        concat_in = [
            jax.device_put(
                np.concatenate([np.asarray(m[nm]) for m in in_maps], axis=0), sh
            )
            for nm in in_names
        ]
        if state["prev"] is None:
            donated = [
                jax.device_put(
                    np.zeros((N_CORES * z.shape[0],) + z.shape[1:], z.dtype), sh
                )
                for z in zero_outs
            ]
        else:
            # gout is fully written by the NEFF, so last call's outputs are
            # valid donation fodder -- avoids re-uploading zeros every call.
            donated = state["prev"]
        state["prev"] = None  # consumed by donation even if the call fails
        out_arrs = sharded(*concat_in, *donated)
        state["prev"] = list(out_arrs)
        host = [np.asarray(a) for a in out_arrs]
        return [
            {nm: host[i].reshape(N_CORES, *out_avals[i].shape)[c]
             for i, nm in enumerate(out_names)}
            for c in range(N_CORES)
        ]

    return call


def _run(nc, in_maps):
    from concourse.bass_utils import axon_active

    if axon_active():
        try:
            if "axon_call" not in _CACHE:
                _CACHE["axon_call"] = _make_axon_callable(nc)
            res = _CACHE["axon_call"](in_maps)
            return [r["gout"] for r in res]
        except Exception:  # noqa: BLE001 - fall through to stock path
            pass
    else:
        try:
            return _run_native(nc, in_maps)
        except Exception:  # noqa: BLE001 - fall through to stock path
            pass
    from concourse.bass_utils import run_bass_kernel_spmd

    res = run_bass_kernel_spmd(nc, in_maps, core_ids=list(range(N_CORES)))
    return [r["gout"] for r in res.results]


def kernel(x, wq, wk, trace=False):
    x = np.asarray(x, np.float32)
    wq = np.asarray(wq, np.float32)
    wk = np.asarray(wk, np.float32)
    if "nc" not in _CACHE:
        _CACHE["nc"] = build_nc()
    nc = _CACHE["nc"]

    in_maps = all_core_inputs(x, wq, wk)
    if trace:
        from concourse.bass_utils import run_bass_kernel_spmd

        res = run_bass_kernel_spmd(
            nc, in_maps, core_ids=list(range(N_CORES)), trace=True
        )
        _CACHE["last_result"] = res
        gouts = [r["gout"] for r in res.results]
    else:
        gouts = _run(nc, in_maps)
    return combine(gouts)


# revision 35
# speedup vs baseline: 215.5962x; 10.7854x over previous
"""Trainium2 Bass kernel: gradient of the EnergyAttention scalar energy.

reference:
    q = einsum('bqd,hzd->bqhz', g, wq); k = einsum('bkd,hzd->bkhz', g, wk)
    scores = einsum('bqhz,bkhz->bhqk', q, k)
    E = -(logsumexp(BETA*scores, -1)/BETA).sum() + POS_SCALE*(g*pos).sum()
    out = dE/dg

Math: with P = softmax(BETA*scores) per (b,h,q):
    out[b] = -sum_h [ (P@K) @ wq_h + (P.T@Qn) @ wk_h ] + POS_SCALE*pos
where Qn = diag(1/Z) Q (row-normalized by the softmax partition Z).

Sharding: 8 cores; core c handles batch b=c//4 and heads 4*(c%4)..4*(c%4)+3
(two head-pairs packed into the 128-partition dim).  Each core takes its
inputs in NATURAL layout (x[b], wq[h0:h0+4], wk[h0:h0+4] -- pure views, no
host-side packing) and emits the positive partial sum_h[(dQ)wq + (dK)wk]
in f32; the host negates, sums the four per-core partials of each batch,
and adds the positional term.

Per-core structure (fused pipeline; P tiles are consumed by the transposed
projection matmuls as soon as they are exp'd, so no full [S,S] matrix is
ever materialized):
  prep:   G^T, wq^T, wk^T built on-device via PE transpose-mode
  proj:   QT2/KT2 [z2, s] = (W GT) via f32r matmuls (d contracted in 8 tiles)
  trans:  Qraw/K2n [s, z2] via PE transpose-mode
  loop i: scores/scoresT blocks (row-tiled K=64 pairs, f32r)
          -> exp on ACT (fp16 out, fused row-sum accum for Z)
          -> dK^T += (Q/Z)^T-block @ P-block, dQ^T += K-block @ PT-block
             (col-tiled M=64 pairs, fp16, single has_written group per bank)
  out:    partial = sum_pairs dQT^T wq + dKT^T wk  (K=128-packed f32r)
"""

import threading

import numpy as np

B = 2
S = 1024
D = 1024
NH = 16
Z = 64
BETA = 1.0 / np.sqrt(np.float32(Z))
POS_SCALE = 0.001
N_CORES = 8
HPC = 4           # heads per core
NPAIR = 2         # head pairs per core
ND = D // 128     # 8 d-tiles
NQ = S // 128     # 8 q/k blocks
NCH = S // 512    # 2 moving-dim chunks

_CACHE = {}


def build_nc(reps=1, taps=(), out_mode="f32copy"):
    """Build the (SPMD, identical-per-core) Bass program.

    reps>1 repeats the whole computation (idempotent) inside one NEFF --
    used to measure steady-state per-execution time as a marginal cost.
    taps: debug-only -- names of internal tiles to DMA out as extra outputs.
    out_mode: "f32copy" (default -- positive partial, host negates; the
    baseline-proven evacuation).  fp16 external outputs are corrupted by a
    f32->f16 conversion misapplied to packed f16 pairs somewhere in the
    output plumbing (odd columns bit-exact, even columns zeroed/exponent-
    shifted), so "f16neg"/"f16copy" are debug-only."""
    from contextlib import ExitStack

    import concourse.mybir as mybir
    import concourse.tile as tile
    from concourse import bacc
    from concourse.masks import make_identity

    F32 = mybir.dt.float32
    F32R = mybir.dt.float32r
    F16 = mybir.dt.float16
    MUL = mybir.AluOpType.mult
    EXP = mybir.ActivationFunctionType.Exp
    COPY = mybir.ActivationFunctionType.Copy

    nc = bacc.Bacc(
        "TRN2",
        target_bir_lowering=False,
        debug=False,
        enable_asserts=False,
        num_devices=N_CORES,
    )

    OUT_DT = F32 if out_mode.startswith("f32") else F16
    xb = nc.dram_tensor("xb", [S, D], F32R, kind="ExternalInput").ap()
    w4q = nc.dram_tensor("w4q", [NPAIR * 128, D], F32R, kind="ExternalInput").ap()
    w4k = nc.dram_tensor("w4k", [NPAIR * 128, D], F32R, kind="ExternalInput").ap()
    gout = nc.dram_tensor("gout", [S, D], OUT_DT, kind="ExternalOutput").ap()

    with tile.TileContext(nc) as tc, ExitStack() as ctx:
        sb1 = ctx.enter_context(tc.tile_pool(name="sb1", bufs=1))
        sb2 = ctx.enter_context(tc.tile_pool(name="sb2", bufs=2))
        sb4 = ctx.enter_context(tc.tile_pool(name="sb4", bufs=4))
        pp = ctx.enter_context(tc.tile_pool(name="pp", bufs=8))
        # PSUM: "sc" 2x[128,1024] (4 banks) shared by prep-transposes/proj/
        # qk-transposes/scores/scoresT/outproj; "d" 2x[128,1024] (4 banks)
        # for the dK then dQ accumulators (each head's accumulator owns a
        # whole tile so each has_written group has its own banks) -> 8 banks.
        ps_sc = ctx.enter_context(tc.tile_pool(name="ps_sc", bufs=2, space="PSUM"))
        ps_d = ctx.enter_context(tc.tile_pool(name="ps_d", bufs=2, space="PSUM"))

        ident = sb1.tile([128, 128], F32, tag="ident")
        make_identity(nc, ident[:])
        ident_r = sb1.tile([128, 128], F32R, tag="ident_r")
        nc.vector.tensor_copy(ident_r[:], ident[:])

        for _rep in range(reps):
            # ---- natural-layout input loads ---------------------------------
            wnq = sb1.tile([128, NPAIR * D], F32R, tag="wnq")  # [z2, (pair, d)]
            wnk = sb1.tile([128, NPAIR * D], F32R, tag="wnk")
            for p in range(NPAIR):
                nc.sync.dma_start(wnq[:, p * D : (p + 1) * D], w4q[p * 128 : (p + 1) * 128, :])
                nc.sync.dma_start(wnk[:, p * D : (p + 1) * D], w4k[p * 128 : (p + 1) * 128, :])

            # ---- on-device transposes: W^T [d, z2] and G^T [d, s] -----------
            wtq = sb1.tile([128, NPAIR * ND * 128], F32R, tag="wtq")  # [d, (pair,dt,z2)]
            wtk = sb1.tile([128, NPAIR * ND * 128], F32R, tag="wtk")
            for wn, wt in ((wnq, wtq), (wnk, wtk)):
                for p in range(NPAIR):
                    ps = ps_sc.tile([128, S], F32R, tag="ps_sc", name=f"wtp_{wt.tensor.name}{p}")
                    for dt in range(ND):
                        nc.tensor.transpose(
                            ps[:, dt * 128 : (dt + 1) * 128],
                            wn[:, p * D + dt * 128 : p * D + (dt + 1) * 128],
                            ident_r[:],
                        )
                    nc.vector.tensor_copy(
                        wt[:, p * ND * 128 : (p + 1) * ND * 128], ps[:]
                    )

            gt = sb1.tile([128, ND * S], F32R, tag="gt")  # G^T: [d_in_tile, (dt, s)]
            for i in range(NQ):
                gn = sb2.tile([128, D], F32R, tag="gn", name=f"gn{i}")
                nc.sync.dma_start(gn[:], xb[i * 128 : (i + 1) * 128, :])
                ps = ps_sc.tile([128, S], F32R, tag="ps_sc", name=f"gtp{i}")
                for dt in range(ND):
                    nc.tensor.transpose(
                        ps[:, dt * 128 : (dt + 1) * 128],
                        gn[:, dt * 128 : (dt + 1) * 128],
                        ident_r[:],
                    )
                for dt in range(ND):
                    nc.any.tensor_copy(
                        gt[:, dt * S + i * 128 : dt * S + (i + 1) * 128],
                        ps[:, dt * 128 : (dt + 1) * 128],
                    )

            # persistent across pairs
            dqt2 = sb1.tile([128, NPAIR * S], F32R, tag="dqt2")  # [z2, (pair, q)]
            dkt2 = sb1.tile([128, NPAIR * S], F32R, tag="dkt2")  # [z2, (pair, k)]
            zrowA = sb1.tile([1, S], F32, tag="zrowA")
            zrowB = sb1.tile([1, S], F32, tag="zrowB")
            ztsb = sb1.tile([16, 128], F32, tag="ztsb")

            pending_dq = []

            def emit_dq_burst():
                """dQ^T(unnorm) += K_i^T PT_i over all blocks, then Z-scale."""
                if not pending_dq:
                    return
                PT_a, k2n_a, zbc_ab, pa = pending_dq.pop()
                dq_ps = [
                    ps_d.tile([128, S], F32, tag="ps_d", name=f"dq_ps{pa}_{a}")
                    for a in range(2)
                ]
                for i in range(NQ):
                    for a in range(2):
                        for ch in range(NCH):
                            nc.tensor.matmul(
                                dq_ps[a][a * 64 : (a + 1) * 64, ch * 512 : (ch + 1) * 512],
                                lhsT=k2n_a[:, i * 128 + a * 64 : i * 128 + (a + 1) * 64],
                                rhs=PT_a[:, (a * NQ + i) * S + ch * 512 : (a * NQ + i) * S + ch * 512 + 512],
                                start=(i == 0),
                                stop=(i == NQ - 1),
                            )
                for a in range(2):
                    nc.vector.tensor_tensor(
                        dqt2[a * 64 : (a + 1) * 64, pa * S : (pa + 1) * S],
                        dq_ps[a][a * 64 : (a + 1) * 64, :],
                        zbc_ab[a][a * 64 : (a + 1) * 64, :],
                        MUL,
                    )

            for p in range(NPAIR):
                # ---- projections: QT2/KT2 [z2, s] ----------------------------
                qt2 = sb2.tile([128, S], F32R, tag="qt2")
                kt2 = sb2.tile([128, S], F32R, tag="kt2")
                for wt, dst in ((wtq, qt2), (wtk, kt2)):
                    ps = ps_sc.tile([128, S], F32, tag="ps_sc", name=f"pj{p}_{dst.tensor.name}")
                    for dt in range(ND):
                        j = p * ND + dt
                        for ch in range(NCH):
                            nc.tensor.matmul(
                                ps[:, ch * 512 : (ch + 1) * 512],
                                lhsT=wt[:, j * 128 : (j + 1) * 128],
                                rhs=gt[:, dt * S + ch * 512 : dt * S + ch * 512 + 512],
                                start=(dt == 0),
                                stop=(dt == ND - 1),
                            )
                    for ch in range(NCH):
                        nc.vector.tensor_copy(
                            dst[:, ch * 512 : (ch + 1) * 512],
                            ps[:, ch * 512 : (ch + 1) * 512],
                        )

                # ---- natural-layout transposes: Qraw / K2n [s, z2] -----------
                qraw = sb2.tile([128, S], F16, tag="qraw")
                k2n = sb2.tile([128, S], F16, tag="k2n")
                for src, dst in ((qt2, qraw), (kt2, k2n)):
                    ps = ps_sc.tile([128, S], F32R, tag="ps_sc", name=f"tr{p}_{dst.tensor.name}")
                    for i in range(NQ):
                        nc.tensor.transpose(
                            ps[:, i * 128 : (i + 1) * 128],
                            src[:, i * 128 : (i + 1) * 128],
                            ident_r[:],
                        )
                    nc.vector.tensor_copy(dst[:], ps[:])

                # previous pair's deferred dQ^T burst: emitted after this pair's
                # proj/transposes so the new scores/exps win scheduler priority
                emit_dq_burst()

                # ---- fused scores/exp/accumulate loop ------------------------
                zsum2 = sb2.tile([128, 16], F32, tag="zsum2")  # [(q), (head, qb)]
                dk_ps = [
                    ps_d.tile([128, S], F32, tag="ps_d", name=f"dk_ps{p}_{a}")
                    for a in range(2)
                ]
                PT_all = pp.tile([128, 2 * NQ * S], F16, tag="PT", bufs=1, name=f"PT{p}")
                for i in range(NQ):
                    # scores blocks [q_i, k] for both heads (row-tiled pairs)
                    pt_s = []
                    for a in range(2):
                        ps = ps_sc.tile([128, S], F32, tag="ps_sc", name=f"sc{p}_{i}_{a}")
                        for ch in range(NCH):
                            nc.tensor.matmul(
                                ps[:, ch * 512 : (ch + 1) * 512],
                                lhsT=qt2[a * 64 : (a + 1) * 64, i * 128 : (i + 1) * 128],
                                rhs=kt2[a * 64 : (a + 1) * 64, ch * 512 : (ch + 1) * 512],
                                start=True,
                                stop=True,
                            )
                        pt_s.append(ps)
                    # P blocks + Z row-sums
                    P_t = []
                    for a in range(2):
                        pb = pp.tile([128, S], F16, tag="P", name=f"P{p}_{i}_{a}")
                        nc.scalar.activation(
                            pb[:],
                            pt_s[a][:],
                            EXP,
                            scale=float(BETA),
                            accum_out=zsum2[:, a * NQ + i : a * NQ + i + 1],
                        )
                        P_t.append(pb)
                    # Qn block = Qraw_i / Z_i
                    q2n_t = sb4.tile([128, 128], F16, tag="q2n", name=f"q2n{p}_{i}")
                    for a in range(2):
                        zq = sb4.tile([128, 1], F32, tag="zq", name=f"zq{p}_{i}_{a}")
                        nc.vector.reciprocal(zq[:], zsum2[:, a * NQ + i : a * NQ + i + 1])
                        nc.vector.tensor_scalar_mul(
                            q2n_t[:, a * 64 : (a + 1) * 64],
                            qraw[:, i * 128 + a * 64 : i * 128 + (a + 1) * 64],
                            zq[:],
                        )
                    # dK^T += Qn_i^T P_i (col-tiled pair; each head's
                    # accumulator owns its own psum tile/banks)
                    for a in range(2):
                        for ch in range(NCH):
                            nc.tensor.matmul(
                                dk_ps[a][a * 64 : (a + 1) * 64, ch * 512 : (ch + 1) * 512],
                                lhsT=q2n_t[:, a * 64 : (a + 1) * 64],
                                rhs=P_t[a][:, ch * 512 : (ch + 1) * 512],
                                start=(i == 0),
                                stop=(i == NQ - 1),
                            )
                    # scoresT blocks [k_i, q] and PT
                    st_s = []
                    for a in range(2):
                        ps = ps_sc.tile([128, S], F32, tag="ps_sc", name=f"st{p}_{i}_{a}")
                        for ch in range(NCH):
                            nc.tensor.matmul(
                                ps[:, ch * 512 : (ch + 1) * 512],
                                lhsT=kt2[a * 64 : (a + 1) * 64, i * 128 : (i + 1) * 128],
                                rhs=qt2[a * 64 : (a + 1) * 64, ch * 512 : (ch + 1) * 512],
                                start=True,
                                stop=True,
                            )
                        st_s.append(ps)
                    for a in range(2):
                        j = a * NQ + i
                        nc.scalar.activation(
                            PT_all[:, j * S : (j + 1) * S], st_s[a][:], EXP, scale=float(BETA)
                        )

                # ---- Z^-1 broadcast [z2, q] then evacuate accumulators -------
                zinv2 = sb2.tile([128, 16], F32, tag="zinv2")
                nc.vector.reciprocal(zinv2[:], zsum2[:])
                zt_ps = ps_sc.tile([128, S], F32, tag="ps_sc", name=f"ztp{p}")
                nc.tensor.transpose(zt_ps[0:16, 0:128], zinv2[:], ident[:])
                nc.vector.tensor_copy(ztsb[:], zt_ps[0:16, 0:128])
                nc.sync.dma_start(
                    zrowA[:].rearrange("p (b c) -> p b c", b=NQ), ztsb[0:NQ, :]
                )
                nc.sync.dma_start(
                    zrowB[:].rearrange("p (b c) -> p b c", b=NQ), ztsb[NQ : 2 * NQ, :]
                )
                # partition_broadcast is only correct to base partition 0 ->
                # broadcast each head's Z row across a full tile, read halves.
                zbcA = sb2.tile([128, S], F32, tag="zbcA")
                zbcB = sb2.tile([128, S], F32, tag="zbcB")
                nc.gpsimd.partition_broadcast(zbcA[:], zrowA[:])
                nc.gpsimd.partition_broadcast(zbcB[:], zrowB[:])

                for a in range(2):
                    nc.vector.tensor_copy(
                        dkt2[a * 64 : (a + 1) * 64, p * S : (p + 1) * S],
                        dk_ps[a][a * 64 : (a + 1) * 64, :],
                    )

                # (the dQ^T burst for this pair is emitted lazily -- see
                # emit_dq_burst -- so the next pair's scores/exps get priority)
                pending_dq.append((PT_all, k2n, (zbcA, zbcB), p))

            emit_dq_burst()

            for tname, tt in (("wnq", wnq), ("wnk", wnk), ("wtq", wtq),
                              ("wtk", wtk), ("gt", gt), ("dqt2", dqt2),
                              ("dkt2", dkt2)):
                if tname in taps:
                    dshape = [tt.tensor.shape[0], tt.tensor.shape[1]]
                    dto = nc.dram_tensor(
                        f"tap_{tname}", dshape, tt.tensor.dtype, kind="ExternalOutput"
                    ).ap()
                    nc.sync.dma_start(dto[:, :], tt[:])

            # ---- output projection  -(sum_h dQ wq + dK wk) -------------------
            for sb in range(NQ):
                ps = ps_sc.tile([128, S], F32, tag="ps_sc", name=f"op{sb}")
                for ch in range(NCH):
                    n = 0
                    for p in range(NPAIR):
                        for dmat, wmat in ((dqt2, wnq), (dkt2, wnk)):
                            nc.tensor.matmul(
                                ps[:, ch * 512 : (ch + 1) * 512],
                                lhsT=dmat[:, p * S + sb * 128 : p * S + (sb + 1) * 128],
                                rhs=wmat[:, p * D + ch * 512 : p * D + ch * 512 + 512],
                                start=(n == 0),
                                stop=(n == 2 * NPAIR - 1),
                            )
                            n += 1
                go = sb4.tile([128, S], OUT_DT, tag="go", name=f"go{sb}")
                if out_mode.endswith("copy"):
                    nc.vector.tensor_copy(go[:], ps[:])
                else:
                    nc.scalar.activation(go[:], ps[:], COPY, scale=-1.0)
                nc.sync.dma_start(gout[sb * 128 : (sb + 1) * 128, :], go[:])

    nc.compile()
    return nc


def core_inputs(x, wq, wk, core):
    """Per-core input arrays -- natural layouts, pure views (no host work)."""
    b = core // 4
    h0 = 4 * (core % 4)
    return {
        "xb": x[b],
        "w4q": wq[h0 : h0 + 4].reshape(NPAIR * 128, D),
        "w4k": wk[h0 : h0 + 4].reshape(NPAIR * 128, D),
    }


def all_core_inputs(x, wq, wk):
    """in_maps for all cores; the x[b] view object is shared across the 4
    cores of batch b so the native path serializes it once per call."""
    xv = [x[b] for b in range(B)]
    maps = []
    for c in range(N_CORES):
        h0 = 4 * (c % 4)
        maps.append(
            {
                "xb": xv[c // 4],
                "w4q": wq[h0 : h0 + 4].reshape(NPAIR * 128, D),
                "w4k": wk[h0 : h0 + 4].reshape(NPAIR * 128, D),
            }
        )
    return maps


def combine(gouts):
    """Host unshard: negate+sum the per-batch positive partials, add pos.

    Memory-bandwidth bound, so split across threads (numpy releases the GIL
    for the large elementwise ops)."""
    pos = np.linspace(-0.5, 0.5, S, dtype=np.float32)[:, None] * np.float32(POS_SCALE)
    out = np.empty((B, S, D), np.float32)
    nchunk = 2
    cs = S // nchunk

    def one(b, j):
        sl = slice(j * cs, (j + 1) * cs)
        acc = gouts[4 * b][sl] + gouts[4 * b + 1][sl]
        np.add(acc, gouts[4 * b + 2][sl], out=acc)
        np.add(acc, gouts[4 * b + 3][sl], out=acc)
        np.subtract(pos[sl], acc, out=out[b][sl])

    ts = [
        threading.Thread(target=one, args=(b, j))
        for b in range(B)
        for j in range(nchunk)
    ]
    for t in ts:
        t.start()
    for t in ts:
        t.join()
    return out


def _out_specs(nc):
    """(name, shape, np-dtype) for each ExternalOutput of nc."""
    import concourse.mybir as mybir

    specs = []
    for alloc in nc.m.functions[0].allocations:
        if isinstance(alloc, mybir.MemoryLocationSet) and alloc.kind == "ExternalOutput":
            specs.append(
                (
                    alloc.memorylocations[0].name,
                    tuple(alloc.tensor_shape),
                    mybir.dt.np(alloc.dtype),
                )
            )
    return specs


def _native_make_sets(st, nc, core, sizes):
    """Persistent per-core NRT tensor sets (allocated once, reused per call)."""
    nrt = st["nrt"]
    ffi, lib = nrt.ffi, nrt.lib
    from concourse.libnrt import deref

    sets = {}
    for kind, names in (("in", st["in_names"]), ("out", [o[0] for o in st["outs"]])):
        set_ptr = ffi.new("nrt_tensor_set_t **")
        ret = lib.nrt_allocate_tensor_set(set_ptr)
        nrt.check_status(ret, "tensor set alloc failed")
        tensors = {}
        for name in names:
            t_ptr = ffi.new("nrt_tensor_t **")
            ret = lib.nrt_tensor_allocate(
                lib.NRT_TENSOR_PLACEMENT_DEVICE,
                core,
                sizes[name],
                name.encode(),
                t_ptr,
            )
            nrt.check_status(ret, f"tensor alloc {name} failed")
            ret = lib.nrt_add_tensor_to_tensor_set(
                deref(set_ptr), name.encode(), deref(t_ptr)
            )
            nrt.check_status(ret, f"add {name} to set failed")
            tensors[name] = (t_ptr, deref(t_ptr))
        sets[kind] = (set_ptr, tensors)
    return sets


def _run_native(nc, in_maps):
    """Fast native path: compile once, keep NEFF loaded and device tensor
    sets allocated across calls."""
    st = _CACHE.get("native")
    if st is None:
        import tempfile

        import concourse.mybir as mybir
        from concourse.bass_utils import compile_bass_kernel, initialize_nrt
        from concourse.libnrt import Krt

        tmpdir = tempfile.mkdtemp()
        neff_file = compile_bass_kernel(nc, tmpdir)
        nrt = initialize_nrt(has_collectives=nc.has_collectives)
        clients = []
        for c in range(N_CORES):
            k = Krt(nrt, core_id=c)
            k.load_model(
                neff_file, cc_enabled=nc.has_collectives, device_count=N_CORES
            )
            clients.append(k)
        part_name = (
            nc.partition_id_tensor.name if nc.partition_id_tensor else None
        )
        in_names, sizes = [], {}
        for alloc in nc.m.functions[0].allocations:
            if not isinstance(alloc, mybir.MemoryLocationSet):
                continue
            name = alloc.memorylocations[0].name
            nbytes = int(np.prod(alloc.tensor_shape)) * mybir.dt.size(alloc.dtype)
            if alloc.kind == "ExternalInput":
                in_names.append(name)
                sizes[name] = nbytes
            elif alloc.kind == "ExternalOutput":
                sizes[name] = nbytes
        st = {
            "nrt": nrt,
            "clients": clients,
            "outs": _out_specs(nc),
            "in_names": in_names,
            "part_name": part_name,
            "sizes": sizes,
        }
        try:
            st["sets"] = [
                _native_make_sets(st, nc, c, sizes) for c in range(N_CORES)
            ]
        except Exception:  # noqa: BLE001
            st["sets"] = None
        _CACHE["native"] = st

    part_name = st["part_name"]
    outs = [None] * N_CORES
    errs = []
    nrt = st["nrt"]

    # Contiguous per-core input arrays (views are already contiguous, so no
    # copies); the fast path writes them to the device with no host staging.
    arr_maps = []
    for c in range(N_CORES):
        m = {k2: np.ascontiguousarray(v) for k2, v in in_maps[c].items()}
        if part_name is not None:
            if "part_arrs" not in st:
                st["part_arrs"] = [
                    np.array([[i]], dtype=np.uint32) for i in range(N_CORES)
                ]
            m[part_name] = st["part_arrs"][c]
        arr_maps.append(m)

    def work_fast(c):
        from concourse.libnrt import deref

        lib, ffi = nrt.lib, nrt.ffi
        sets = st["sets"][c]
        in_set, in_tensors = sets["in"]
        out_set, out_tensors = sets["out"]
        for name, arr in arr_maps[c].items():
            t = in_tensors[name][1]
            ret = lib.nrt_tensor_write(t, ffi.from_buffer(arr), 0, arr.nbytes)
            nrt.check_status(ret, f"tensor write {name} failed")
        model = st["clients"][c].nrt_models[0]
        ret = lib.nrt_execute(model, deref(in_set), deref(out_set))
        nrt.check_status(ret, "nrt_execute failed")
        nm, shp, dt = st["outs"][0]
        out_arr = np.empty(shp, dt)
        ret = lib.nrt_tensor_read(
            out_tensors[nm][1],
            ffi.from_buffer(out_arr, require_writable=True),
            0,
            out_arr.nbytes,
        )
        nrt.check_status(ret, f"tensor read {nm} failed")
        outs[c] = out_arr

    def work(c):
        try:
            if st["sets"] is not None:
                try:
                    work_fast(c)
                    return
                except Exception:  # noqa: BLE001
                    pass
            ser_map = {k2: v.tobytes() for k2, v in arr_maps[c].items()}
            outputs_c = {
                nm: np.zeros(shp, dt).tobytes() for nm, shp, dt in st["outs"]
            }
            st["clients"][c].model_execute(0, ser_map, outputs_c)
            nm, shp, dt = st["outs"][0]
            outs[c] = np.frombuffer(outputs_c[nm], dt).reshape(shp)
        except Exception as e:  # noqa: BLE001
            errs.append(e)

    threads = [threading.Thread(target=work, args=(c,)) for c in range(N_CORES)]
    for t in threads:
        t.start()
    for t in threads:
        t.join()
    if errs:
        raise errs[0]
    return outs


def _make_axon_callable(nc):
    """Persistent jit'd sharded runner for the axon (tunneled-PJRT) path."""
    import jax

    try:
        from jax.experimental.shard_map import shard_map as _smap

        def shard_map(f, mesh, in_specs, out_specs):
            return _smap(
                f, mesh=mesh, in_specs=in_specs, out_specs=out_specs,
                check_rep=False,
            )
    except ImportError:  # jax >= 0.9: experimental alias removed

        def shard_map(f, mesh, in_specs, out_specs):
            return jax.shard_map(
                f, mesh=mesh, in_specs=in_specs, out_specs=out_specs,
                check_vma=False,
            )

    from jax.sharding import Mesh, NamedSharding, PartitionSpec

    import concourse.mybir as mybir
    from concourse import bass2jax

    bass2jax.install_neuronx_cc_hook()
    partition_name = nc.partition_id_tensor.name if nc.partition_id_tensor else None
    in_names, out_names, out_avals, zero_outs = [], [], [], []
    for alloc in nc.m.functions[0].allocations:
        if not isinstance(alloc, mybir.MemoryLocationSet):
            continue
        name = alloc.memorylocations[0].name
        if alloc.kind == "ExternalInput":
            if name != partition_name:
                in_names.append(name)
        elif alloc.kind == "ExternalOutput":
            shape = tuple(alloc.tensor_shape)
            dtype = mybir.dt.np(alloc.dtype)
            out_names.append(name)
            out_avals.append(jax.core.ShapedArray(shape, dtype))
            zero_outs.append(np.zeros(shape, dtype))
    n_params = len(in_names)
    all_in_names = list(in_names) + out_names
    if partition_name is not None:
        all_in_names.append(partition_name)

    def _body(*args):
        operands = list(args)
        if partition_name is not None:
            operands.append(bass2jax.partition_id_tensor())
        outs = bass2jax._bass_exec_p.bind(
            *operands,
            out_avals=tuple(out_avals),
            in_names=tuple(all_in_names),
            out_names=tuple(out_names),
            lowering_input_output_aliases=(),
            sim_require_finite=True,
            sim_require_nnan=True,
            nc=nc,
        )
        return tuple(outs)

    devices = jax.devices()[:N_CORES]
    mesh = Mesh(np.asarray(devices), ("core",))
    spec = PartitionSpec("core")
    sharded = jax.jit(
        shard_map(
            _body,
            mesh,
            (spec,) * (n_params + len(out_names)),
            (spec,) * len(out_names),
        ),
        donate_argnums=tuple(range(n_params, n_params + len(out_names))),
        keep_unused=True,
    )
    sh = NamedSharding(mesh, spec)
    state = {"prev": None}

    def call(in_maps):
        concat_in = [
            jax.device_put(
                np.concatenate([np.asarray(m[nm]) for m in in_maps], axis=0), sh
            )
            for nm in in_names
        ]
        if state["prev"] is None:
            donated = [
                jax.device_put(
                    np.zeros((N_CORES * z.shape[0],) + z.shape[1:], z.dtype), sh
                )
                for z in zero_outs
            ]
        else:
            # gout is fully written by the NEFF, so last call's outputs are
            # valid donation fodder -- avoids re-uploading zeros every call.
            donated = state["prev"]
        state["prev"] = None  # consumed by donation even if the call fails
        out_arrs = sharded(*concat_in, *donated)
        state["prev"] = list(out_arrs)
        host = [np.asarray(a) for a in out_arrs]
        return [
            {nm: host[i].reshape(N_CORES, *out_avals[i].shape)[c]
             for i, nm in enumerate(out_names)}
            for c in range(N_CORES)
        ]

    return call


def _run(nc, in_maps):
    from concourse.bass_utils import axon_active

    if axon_active():
        try:
            if "axon_call" not in _CACHE:
                _CACHE["axon_call"] = _make_axon_callable(nc)
            res = _CACHE["axon_call"](in_maps)
            return [r["gout"] for r in res]
        except Exception:  # noqa: BLE001 - fall through to stock path
            pass
    else:
        try:
            return _run_native(nc, in_maps)
        except Exception:  # noqa: BLE001 - fall through to stock path
            pass
    from concourse.bass_utils import run_bass_kernel_spmd

    res = run_bass_kernel_spmd(nc, in_maps, core_ids=list(range(N_CORES)))
    return [r["gout"] for r in res.results]


def kernel(x, wq, wk, trace=False):
    x = np.asarray(x, np.float32)
    wq = np.asarray(wq, np.float32)
    wk = np.asarray(wk, np.float32)
    if "nc" not in _CACHE:
        _CACHE["nc"] = build_nc()
    nc = _CACHE["nc"]

    in_maps = all_core_inputs(x, wq, wk)
    if trace:
        from concourse.bass_utils import run_bass_kernel_spmd

        res = run_bass_kernel_spmd(
            nc, in_maps, core_ids=list(range(N_CORES)), trace=True
        )
        _CACHE["last_result"] = res
        gouts = [r["gout"] for r in res.results]
    else:
        gouts = _run(nc, in_maps)
    return combine(gouts)
